# revision 1
# baseline (speedup 1.0000x reference)
"""ChebNet (K=3, 2 layers) message-passing kernel for 8 Trainium2 NeuronCores.

Strategy (dest-sharded, degree-sorted column gather with CCE accumulation):
  - Nodes are sharded across 8 cores by contiguous id range (12500 each).
  - Within a core, destination nodes are sorted by in-degree (descending) and
    laid out in 98 "columns" of 128 nodes (rank r -> column r//128,
    partition r%128).  All node-indexed device state lives in this order.
  - The propagation  L_hat @ t = -D^-1/2 A D^-1/2 t  is factored as
    -d * (A @ (d * t)), so per-edge weights disappear: only unweighted
    gather+sum remains, plus cheap per-node scalings (folded into the
    Chebyshev/weight algebra below, so only *positive* d multiplies happen
    on device and all signs/factors live in host-prepared weight matrices).
  - A @ ts is computed with chains of `indirect_dma_start` gathers with
    compute_op=add: pass k of column b fetches the k-th edge's source row for
    each of the 128 dests of that column and CCE-accumulates into the
    column's SBUF accumulator.  Because columns hold equal-degree dests, the
    number of passes per column is (max) degree in that column, and total
    calls/prop is within ~2.5% of edges/128.  Pad slots fetch a zero row.
  - Full (scaled) node features for gathering are replicated on every core
    via AllGather between propagations.
  - Feature-major copies for the matmuls are produced with PE transposes;
    ChebConv matmuls run with K packed to 128 ([T0;T1] against [W0-W2;-W1])
    plus a K=64 term (2*W2) accumulated in PSUM.

Self-contained: hardcodes the problem shapes; builds indices from the given
edge_index on the host (numpy), compiles one SPMD Bass program, runs it on
cores 0..7, and reassembles the full [100000, 32] output.
"""

import numpy as np

N_NODES = 100_000
N_EDGES = 1_200_000
CIN, CHID, COUT, KCH = 64, 64, 32, 3
NCORES = 8
NLOC = N_NODES // NCORES            # 12500
NCOLS = (NLOC + 127) // 128         # 98
PADLOC = NCOLS * 128                # 12544
TSTRIDE = PADLOC + 16               # per-core block in gather tables
ZROW = PADLOC                       # a zero row (core 0's pad rows)
NTAB = TSTRIDE * NCORES             # 100480

_CACHE = {}


# --------------------------------------------------------------------------
# Host-side index construction
# --------------------------------------------------------------------------

def _prep_indices(edge_index):
    row = np.asarray(edge_index[0], dtype=np.int64)
    col = np.asarray(edge_index[1], dtype=np.int64)
    deg = np.bincount(row, minlength=N_NODES)
    d = np.where(deg > 0, 1.0 / np.sqrt(np.maximum(deg, 1)), 0.0).astype(np.float32)

    # per-core degree-descending permutation
    owner = np.arange(N_NODES) // NLOC
    rank = np.empty(N_NODES, dtype=np.int64)
    lnode = np.empty((NCORES, NLOC), dtype=np.int64)
    for c in range(NCORES):
        ids = np.arange(c * NLOC, (c + 1) * NLOC)
        order = np.argsort(-deg[ids], kind="stable")
        lnode[c] = ids[order]
        rank[ids[order]] = np.arange(NLOC)
    pos = owner * TSTRIDE + rank     # table position of each node

    # column degree profile per core and unified schedule
    D_cols = np.zeros((NCORES, NCOLS), dtype=np.int64)
    for c in range(NCORES):
        dl = deg[lnode[c]]
        dl = np.concatenate([dl, np.zeros(PADLOC - NLOC, dtype=np.int64)])
        D_cols[c] = dl.reshape(NCOLS, 128).max(axis=1)
    D_sched = D_cols.max(axis=0)     # [NCOLS]
    maxk = int(D_sched.max())

    # call schedule, k-major (so all column chains progress together)
    calls = []                       # (b, k)
    call_id = -np.ones((NCOLS, maxk), dtype=np.int64)
    for k in range(maxk):
        for b in range(NCOLS):
            if D_sched[b] > k:
                call_id[b, k] = len(calls)
                calls.append((b, k))
    ncalls = len(calls)

    # per-core gather index tables  gidx[core][p, call] = table pos of source
    gidx = np.full((NCORES, 128, ncalls), ZROW, dtype=np.int32)
    # order edges by (dest position, occurrence)
    dpos = pos[row]
    order = np.argsort(dpos, kind="stable")
    dpos_s = dpos[order]
    spos_s = pos[col][order].astype(np.int32)
    # occurrence index within each dest group
    first = np.ones(len(dpos_s), dtype=bool)
    first[1:] = dpos_s[1:] != dpos_s[:-1]
    grp_start = np.maximum.accumulate(np.where(first, np.arange(len(dpos_s)), 0))
    occ = np.arange(len(dpos_s)) - grp_start
    core_e = dpos_s // TSTRIDE
    r_e = dpos_s % TSTRIDE
    b_e = r_e // 128
    p_e = r_e % 128
    cid = call_id[b_e, occ]
    assert (cid >= 0).all()
    gidx[core_e, p_e, cid] = spos_s

    dloc = np.zeros((NCORES, PADLOC), dtype=np.float32)
    for c in range(NCORES):
        dloc[c, :NLOC] = d[lnode[c]]

    return {
        "lnode": lnode, "gidx": gidx, "calls": calls, "ncalls": ncalls,
        "dloc": dloc, "d": d,
    }


# --------------------------------------------------------------------------
# Device program
# --------------------------------------------------------------------------

def _build_program(calls):
    from concourse import bass, bacc, tile, mybir

    ncalls = len(calls)
    f32 = mybir.dt.float32
    nc = bacc.Bacc("TRN2", target_bir_lowering=False, debug=False,
                   num_devices=NCORES)

    ts0 = nc.dram_tensor("ts0", [NTAB, CIN], f32, kind="ExternalInput")
    xfm = nc.dram_tensor("xfm", [CIN, PADLOC], f32, kind="ExternalInput")
    dnm_in = nc.dram_tensor("dnm", [128, NCOLS * 64], f32, kind="ExternalInput")
    gidx_in = nc.dram_tensor("gidx", [128, ncalls], mybir.dt.int32,
                             kind="ExternalInput")
    w1a_in = nc.dram_tensor("w1a", [128, CHID], f32, kind="ExternalInput")
    w1b_in = nc.dram_tensor("w1b", [64, CHID], f32, kind="ExternalInput")
    w2a_in = nc.dram_tensor("w2a", [128, COUT], f32, kind="ExternalInput")
    w2b_in = nc.dram_tensor("w2b", [64, COUT], f32, kind="ExternalInput")
    b1_in = nc.dram_tensor("b1v", [CHID, 1], f32, kind="ExternalInput")
    b2_in = nc.dram_tensor("b2v", [COUT, 1], f32, kind="ExternalInput")
    id_in = nc.dram_tensor("ident", [128, 128], f32, kind="ExternalInput")
    out_d = nc.dram_tensor("out", [PADLOC, COUT], f32, kind="ExternalOutput")

    Relu = mybir.ActivationFunctionType.Relu
    Ident = mybir.ActivationFunctionType.Identity
    MUL = mybir.AluOpType.mult
    ADD = mybir.AluOpType.add
    BYP = mybir.AluOpType.bypass

    with tile.TileContext(nc) as tc:
        with (
            tc.tile_pool(name="consts", bufs=1) as consts,
            tc.tile_pool(name="accp", bufs=1) as accp,
            tc.tile_pool(name="fmp", bufs=1) as fmp,
            tc.tile_pool(name="tmpp", bufs=6) as tmpp,
            tc.tile_pool(name="fm2p", bufs=3) as fm2p,
            tc.tile_pool(name="ofmp", bufs=3) as ofmp,
            tc.tile_pool(name="psT", bufs=3, space="PSUM") as psT,
            tc.tile_pool(name="psM", bufs=2, space="PSUM") as psM,
            tc.tile_pool(name="psH", bufs=3, space="PSUM") as psH,
            tc.tile_pool(name="dram", bufs=1, space="DRAM") as dram,
        ):
            # ---- constants into SBUF
            gidx_sb = consts.tile([128, ncalls], mybir.dt.int32)
            nc.sync.dma_start(out=gidx_sb[:], in_=gidx_in[:])
            dnm = consts.tile([128, NCOLS, 64], f32)
            nc.sync.dma_start(out=dnm[:], in_=dnm_in[:].rearrange("p (b c) -> p b c", b=NCOLS))
            w1a = consts.tile([128, CHID], f32)
            nc.sync.dma_start(out=w1a[:], in_=w1a_in[:])
            w1b = consts.tile([64, CHID], f32)
            nc.sync.dma_start(out=w1b[:], in_=w1b_in[:])
            w2a = consts.tile([128, COUT], f32)
            nc.sync.dma_start(out=w2a[:], in_=w2a_in[:])
            w2b = consts.tile([64, COUT], f32)
            nc.sync.dma_start(out=w2b[:], in_=w2b_in[:])
            b1v = consts.tile([CHID, 1], f32)
            nc.sync.dma_start(out=b1v[:], in_=b1_in[:])
            b2v = consts.tile([COUT, 1], f32)
            nc.sync.dma_start(out=b2v[:], in_=b2_in[:])
            ident = consts.tile([128, 128], f32)
            nc.sync.dma_start(out=ident[:], in_=id_in[:])
            zrow = consts.tile([16, 64], f32)
            nc.gpsimd.memset(zrow[:], 0.0)

            # fmA: partitions 0:64 = Tx0 features (x / h), 64:128 = Tx1_s
            fmA = fmp.tile([128, PADLOC], f32)
            nc.sync.dma_start(out=fmA[0:64, :], in_=xfm[:])
            # node-major staging for AllGather inputs
            tsn = fmp.tile([128, NCOLS, 64], f32)
            # final node-major output staging
            outn = fmp.tile([128, NCOLS, COUT], f32)

            # accumulator columns
            acc = [accp.tile([128, 64], f32, name=f"acc{b}") for b in range(NCOLS)]

            # DRAM: AllGather bounce + tables
            ag_in = dram.tile([TSTRIDE, CIN], f32, name="ag_in")
            tabs = [dram.tile([NTAB, CIN], f32, name=f"tab{i}",
                              addr_space="Shared") for i in range(3)]
            # zero the 16 pad rows of our AG block once; they become the
            # shared zero rows of every table after AllGather
            nc.sync.dma_start(out=ag_in[PADLOC:TSTRIDE, :], in_=zrow[:])

            def prop(table):
                for j, (b, k) in enumerate(calls):
                    nc.gpsimd.indirect_dma_start(
                        out=acc[b][:], out_offset=None, in_=table[:],
                        in_offset=bass.IndirectOffsetOnAxis(
                            ap=gidx_sb[:, j:j + 1], axis=0),
                        compute_op=(BYP if k == 0 else ADD),
                    )

            def scale_to_fm(dst_fm, dst_part0, with_ts):
                """t = d*acc per column; PE-transpose into dst_fm (feature-
                major, partitions dst_part0:dst_part0+64).  If with_ts, also
                write tsn[:, b, :] = d*t (the next gather table slice)."""
                for b in range(NCOLS):
                    t1 = tmpp.tile([128, 64], f32, tag="t1", name=f"t1_{b}")
                    nc.vector.tensor_tensor(out=t1[:], in0=acc[b][:],
                                            in1=dnm[:, b, :], op=MUL)
                    pt = psT.tile([128, 128], f32, tag="pt", name=f"pt_{b}")
                    nc.tensor.transpose(out=pt[0:64, :], in_=t1[:],
                                        identity=ident[:])
                    nc.vector.tensor_copy(
                        out=dst_fm[dst_part0:dst_part0 + 64,
                                   b * 128:(b + 1) * 128],
                        in_=pt[0:64, :])
                    if with_ts:
                        nc.vector.tensor_tensor(out=tsn[:, b, :], in0=t1[:],
                                                in1=dnm[:, b, :], op=MUL)

            def allgather(tab):
                nc.sync.dma_start(
                    out=ag_in[0:PADLOC, :].rearrange("(b p) c -> p b c", p=128),
                    in_=tsn[:])
                nc.gpsimd.collective_compute(
                    "AllGather", BYP,
                    replica_groups=[list(range(NCORES))],
                    ins=[ag_in[:].opt()], outs=[tab[:].opt()])

            # ================= layer 1 =================
            prop(ts0)                     # acc = A @ ts0
            scale_to_fm(fmA, 64, with_ts=True)   # Tx1_s -> fmA[64:], ts1 -> tsn
            allgather(tabs[0])
            prop(tabs[0])                 # acc = A @ ts1

            # L1 matmuls + relu; h overwrites fmA[0:64]; also build ts_h
            ntile = (PADLOC + 511) // 512
            for j in range(ntile):
                j0, j1 = j * 512, min((j + 1) * 512, PADLOC)
                w = j1 - j0
                fm2 = fm2p.tile([64, 512], f32, tag="fm2", name=f"fm2_{j}")
                for bi in range(4):
                    b = j * 4 + bi
                    if b >= NCOLS:
                        break
                    t2 = tmpp.tile([128, 64], f32, tag="t1", name=f"t2_{b}")
                    nc.vector.tensor_tensor(out=t2[:], in0=acc[b][:],
                                            in1=dnm[:, b, :], op=MUL)
                    pt2 = psT.tile([128, 128], f32, tag="pt", name=f"pt2_{b}")
                    nc.tensor.transpose(out=pt2[0:64, :], in_=t2[:],
                                        identity=ident[:])
                    nc.vector.tensor_copy(out=fm2[0:64, bi * 128:(bi + 1) * 128],
                                          in_=pt2[0:64, :])
                pm = psM.tile([64, 512], f32, tag="pm", name=f"pm_{j}")
                nc.tensor.matmul(out=pm[:, :w], lhsT=w1a[:], rhs=fmA[:, j0:j1],
                                 start=True, stop=False)
                nc.tensor.matmul(out=pm[:, :w], lhsT=w1b[:], rhs=fm2[0:64, :w],
                                 start=False, stop=True)
                # h = relu(pm + b1) -> fmA[0:64]
                nc.scalar.activation(fmA[0:64, j0:j1], pm[:, :w], Relu,
                                     bias=b1v[:, 0:1])
                # ts_h = d * h (node-major)
                for bi in range(4):
                    b = j * 4 + bi
                    if b >= NCOLS:
                        break
                    ph = psH.tile([128, 64], f32, tag="ph", name=f"ph_{b}")
                    nc.tensor.transpose(
                        out=ph[:], in_=fmA[0:64, b * 128:(b + 1) * 128],
                        identity=ident[0:64, 0:64])
                    nc.vector.tensor_tensor(out=tsn[:, b, :], in0=ph[:],
                                            in1=dnm[:, b, :], op=MUL)

            # ================= layer 2 =================
            allgather(tabs[1])
            prop(tabs[1])                 # acc = A @ ts_h
            scale_to_fm(fmA, 64, with_ts=True)   # Tx1'_s -> fmA[64:], ts1' -> tsn
            allgather(tabs[2])
            prop(tabs[2])                 # acc = A @ ts1'

            for j in range(ntile):
                j0, j1 = j * 512, min((j + 1) * 512, PADLOC)
                w = j1 - j0
                fm2 = fm2p.tile([64, 512], f32, tag="fm2", name=f"fm2b_{j}")
                for bi in range(4):
                    b = j * 4 + bi
                    if b >= NCOLS:
                        break
                    t2 = tmpp.tile([128, 64], f32, tag="t1", name=f"t2b_{b}")
                    nc.vector.tensor_tensor(out=t2[:], in0=acc[b][:],
                                            in1=dnm[:, b, :], op=MUL)
                    pt2 = psT.tile([128, 128], f32, tag="pt", name=f"pt2b_{b}")
                    nc.tensor.transpose(out=pt2[0:64, :], in_=t2[:],
                                        identity=ident[:])
                    nc.vector.tensor_copy(out=fm2[0:64, bi * 128:(bi + 1) * 128],
                                          in_=pt2[0:64, :])
                pm = psM.tile([64, 512], f32, tag="pm", name=f"pmb_{j}")
                nc.tensor.matmul(out=pm[0:COUT, :w], lhsT=w2a[:],
                                 rhs=fmA[:, j0:j1], start=True, stop=False)
                nc.tensor.matmul(out=pm[0:COUT, :w], lhsT=w2b[:],
                                 rhs=fm2[0:64, :w], start=False, stop=True)
                ofm = ofmp.tile([COUT, 512], f32, tag="ofm", name=f"ofm_{j}")
                nc.scalar.activation(ofm[:, :w], pm[0:COUT, :w], Ident,
                                     bias=b2v[:, 0:1])
                for bi in range(4):
                    b = j * 4 + bi
                    if b >= NCOLS:
                        break
                    po = psH.tile([128, 64], f32, tag="ph", name=f"po_{b}")
                    nc.tensor.transpose(
                        out=po[:, 0:COUT], in_=ofm[:, bi * 128:(bi + 1) * 128],
                        identity=ident[0:COUT, 0:COUT])
                    nc.vector.tensor_copy(out=outn[:, b, :], in_=po[:, 0:COUT])

            nc.sync.dma_start(
                out=out_d[:].rearrange("(b p) c -> p b c", p=128),
                in_=outn[:])

    nc.finalize()
    return nc


# --------------------------------------------------------------------------
# PJRT runner (jit once, reuse)
# --------------------------------------------------------------------------

def _make_runner(nc):
    import jax
    from jax.sharding import Mesh, PartitionSpec
    from jax.experimental.shard_map import shard_map
    from concourse import mybir
    from concourse.bass2jax import (_bass_exec_p, install_neuronx_cc_hook,
                                    partition_id_tensor)

    install_neuronx_cc_hook()
    partition_name = nc.partition_id_tensor.name if nc.partition_id_tensor else None
    in_names, out_names, out_avals = [], [], []
    for alloc in nc.m.functions[0].allocations:
        if not isinstance(alloc, mybir.MemoryLocationSet):
            continue
        name = alloc.memorylocations[0].name
        if alloc.kind == "ExternalInput":
            if name != partition_name:
                in_names.append(name)
        elif alloc.kind == "ExternalOutput":
            out_names.append(name)
            out_avals.append(jax.core.ShapedArray(tuple(alloc.tensor_shape),
                                                  mybir.dt.np(alloc.dtype)))
    n_params = len(in_names)
    all_in = list(in_names) + list(out_names)
    if partition_name is not None:
        all_in.append(partition_name)
    donate = tuple(range(n_params, n_params + len(out_names)))

    def _body(*args):
        operands = list(args)
        if partition_name is not None:
            operands.append(partition_id_tensor())
        return tuple(_bass_exec_p.bind(
            *operands, out_avals=tuple(out_avals), in_names=tuple(all_in),
            out_names=tuple(out_names), lowering_input_output_aliases=(),
            sim_require_finite=True, sim_require_nnan=True, nc=nc))

    devices = jax.devices()[:NCORES]
    mesh = Mesh(np.asarray(devices), ("core",))
    in_specs = (PartitionSpec("core"),) * (n_params + len(out_names))
    out_specs = (PartitionSpec("core"),) * len(out_names)
    fn = jax.jit(shard_map(_body, mesh=mesh, in_specs=in_specs,
                           out_specs=out_specs, check_rep=False),
                 donate_argnums=donate, keep_unused=True)

    state = {"staged": None}

    def stage(in_maps):
        per_core = [[np.asarray(m[n]) for n in in_names] for m in in_maps]
        concat_in = [np.concatenate([per_core[c][i] for c in range(NCORES)],
                                    axis=0) for i in range(n_params)]
        state["staged"] = [jax.device_put(a) for a in concat_in]
        jax.block_until_ready(state["staged"])

    def run():
        import time
        concat_zeros = [np.zeros((NCORES * a.shape[0], *a.shape[1:]), a.dtype)
                        for a in out_avals]
        zs = [jax.device_put(z) for z in concat_zeros]
        jax.block_until_ready(zs)
        t0 = time.time()
        outs = fn(*state["staged"], *zs)
        jax.block_until_ready(outs)
        dt = time.time() - t0
        res = [{n: np.asarray(outs[i]).reshape(NCORES, *out_avals[i].shape)[c]
                for i, n in enumerate(out_names)} for c in range(NCORES)]
        return res, dt

    return stage, run


# --------------------------------------------------------------------------
# Entry point
# --------------------------------------------------------------------------

def _get_compiled(edge_index):
    key = hash(np.asarray(edge_index)[:, :: max(1, N_EDGES // 1024)].tobytes())
    if key in _CACHE:
        return _CACHE[key]
    prep = _prep_indices(edge_index)
    nc = _build_program(prep["calls"])
    stage, run = _make_runner(nc)
    _CACHE[key] = (prep, stage, run)
    return _CACHE[key]


def kernel(x, edge_index, W1, b1, W2, b2):
    x = np.asarray(x, dtype=np.float32)
    W1 = np.asarray(W1, dtype=np.float32)
    W2 = np.asarray(W2, dtype=np.float32)
    b1 = np.asarray(b1, dtype=np.float32)
    b2 = np.asarray(b2, dtype=np.float32)

    prep, stage, run = _get_compiled(edge_index)
    lnode, gidx, dloc, d = (prep["lnode"], prep["gidx"], prep["dloc"],
                            prep["d"])

    # gather table for prop 1: pos-ordered d*x with zero row
    ts0 = np.zeros((NTAB, CIN), dtype=np.float32)
    dx = d[:, None] * x
    for c in range(NCORES):
        ts0[c * TSTRIDE:c * TSTRIDE + NLOC] = dx[lnode[c]]

    w1a = np.concatenate([W1[0] - W1[2], -W1[1]], axis=0)      # [128, 64]
    w1b = 2.0 * W1[2]                                          # [64, 64]
    w2a = np.concatenate([W2[0] - W2[2], -W2[1]], axis=0)      # [128, 32]
    w2b = 2.0 * W2[2]                                          # [64, 32]
    ident = np.eye(128, dtype=np.float32)

    in_maps = []
    for c in range(NCORES):
        xl = np.zeros((PADLOC, CIN), dtype=np.float32)
        xl[:NLOC] = x[lnode[c]]
        dn = dloc[c].reshape(NCOLS, 128).T                     # [128, NCOLS]
        dnm = np.repeat(dn[:, :, None], 64, axis=2).reshape(128, NCOLS * 64)
        in_maps.append({
            "ts0": ts0, "xfm": np.ascontiguousarray(xl.T),
            "dnm": np.ascontiguousarray(dnm), "gidx": gidx[c],
            "w1a": w1a, "w1b": w1b, "w2a": w2a, "w2b": w2b,
            "b1v": b1[:, None], "b2v": b2[:, None], "ident": ident,
        })

    stage(in_maps)
    res, dt = run()
    kernel.last_exec_wall_s = dt
    kernel.rerun = run

    out = np.empty((N_NODES, COUT), dtype=np.float32)
    for c in range(NCORES):
        out[lnode[c]] = res[c]["out"][:NLOC]
    return out

